# revision 5
# baseline (speedup 1.0000x reference)
"""GCN (2-layer graph conv + dense head + softmax) on 8 Trainium2 NeuronCores.

Strategy (dst-sharded graph parallel):
  - 50000 nodes padded to 50176 = 8 * 6272; core c owns local node slots
    [c*6272, (c+1)*6272) (padded global id == real id; pads live at the end).
  - Edges are partitioned by destination owner, grouped by (dst tile of 128
    nodes, src-table half) and cut into 128-edge chunks. The chunk/batch
    structure is made uniform across cores (max over cores) so one SPMD
    program serves all 8 cores.
  - Normalization 1/sqrt(deg) is applied as row scalings: the src side is
    folded into the gathered feature table, the dst side into PSUM
    evacuation (scalings commute with relu for positive scale).
  - Feature tables are stored as interleaved bf16 (hi | lo) splits of the
    fp32 values so the scatter-matmuls run at full bf16 PE rate while keeping
    ~2^-16 relative accuracy. PSUM accumulates in fp32.
  - Per-edge gather uses gpsimd dma_gather (SWDGE row gather, int16 indices,
    table split in two halves of 25088 rows to fit int16).
  - Scatter-add is a one-hot matmul: onehot[e, d] = (dst_in_tile[e] == d),
    accumulated into PSUM per destination tile.
  - deg is computed on-device with the same one-hot matmuls against ones.
  - Between layers the (scaled, hi/lo split) activations are AllGather'd.
"""

import sys

sys.path.insert(0, "/opt/trn_rl_repo")

import numpy as np
import ml_dtypes

import concourse.bass as bass
import concourse.bacc as bacc
import concourse.tile as tile
import concourse.mybir as mybir
from concourse.bass_utils import run_bass_kernel_spmd
from concourse.library_config import mlp

FP32 = mybir.dt.float32
BF16 = mybir.dt.bfloat16
I16 = mybir.dt.int16

NC = 8
P = 128
BATCH = 8  # chunks per dma_gather batch


class Plan:
    """Host-side (index-only) sharding plan + per-core staged arrays."""

    def __init__(self, n_nodes, f_in, h_dim, c_out, edge_index):
        self.N = n_nodes
        self.F = f_in
        self.H = h_dim
        self.C = c_out
        self.NPC = ((n_nodes + NC * P - 1) // (NC * P)) * P
        self.NT = self.NPC // P
        self.NPAD = self.NPC * NC
        self.HALF = self.NPAD // 2
        assert self.HALF < 32768, "int16 gather index limit"

        src = np.asarray(edge_index[0], dtype=np.int64)
        dst = np.asarray(edge_index[1], dtype=np.int64)

        dst_c = np.minimum(dst // self.NPC, NC - 1)
        dst_l = dst - dst_c * self.NPC
        tile_id = dst_l >> 7
        din = dst_l & 127
        half = (src >= self.HALF).astype(np.int64)
        src_h = src - half * self.HALF

        per_core = []
        counts = np.zeros((NC, self.NT, 2), dtype=np.int64)
        for c in range(NC):
            m = dst_c == c
            key = tile_id[m] * 2 + half[m]
            order = np.argsort(key, kind="stable")
            per_core.append((src_h[m][order], din[m][order]))
            cnt = np.bincount(key, minlength=self.NT * 2)
            counts[c] = cnt.reshape(self.NT, 2)

        maxc = counts.max(axis=0)            # [NT, 2]
        ncth = -(-maxc // P)                 # ceil chunk counts
        self.ncth = ncth

        # global chunk list, tile-major; batches assigned per half-stream
        chunks = []          # (tile, half, batch, pos_in_batch)
        nb = [0, 0]
        bpos = [0, 0]
        for t in range(self.NT):
            for h in range(2):
                for _k in range(int(ncth[t, h])):
                    if bpos[h] == 0:
                        nb[h] += 1
                    chunks.append((t, h, nb[h] - 1, bpos[h]))
                    bpos[h] = (bpos[h] + 1) % BATCH
        self.NB0, self.NB1 = nb[0], nb[1]
        self.NB = max(nb[0] + nb[1], 1)
        self.chunks = [
            (t, h, b + (self.NB0 if h == 1 else 0), pos)
            for (t, h, b, pos) in chunks
        ]
        self.NCH = max(len(self.chunks), 1)
        # chunk indices per tile (tile-major => contiguous)
        self.tile_chunks = [[] for _ in range(self.NT)]
        for ci, (t, h, b, pos) in enumerate(self.chunks):
            self.tile_chunks[t].append(ci)

        # per-core staged arrays
        self.dstv = np.full((NC, P, self.NCH), -1.0, dtype=np.float32)
        srcw = np.zeros((NC, self.NB * BATCH * P), dtype=np.int16)
        for c in range(NC):
            e_src, e_din = per_core[c]
            seg_cnt = counts[c].reshape(-1)
            seg_off = np.concatenate([[0], np.cumsum(seg_cnt)])
            used = np.zeros(self.NT * 2, dtype=np.int64)
            for ci, (t, h, b, pos) in enumerate(self.chunks):
                s = t * 2 + h
                lo = seg_off[s] + used[s]
                n = int(min(P, seg_cnt[s] - used[s]))
                if n <= 0:
                    continue
                used[s] += n
                self.dstv[c, :n, ci] = e_din[lo : lo + n].astype(np.float32)
                base = b * BATCH * P + pos * P
                srcw[c, base : base + n] = e_src[lo : lo + n].astype(np.int16)
        # wrapped int16 index layout: [16, NB*64], tiled to [128, NB*64]
        self.srcw = np.zeros((NC, P, self.NB * 64), dtype=np.int16)
        for c in range(NC):
            w = srcw[c].reshape(self.NB, 64, 16)
            w = np.transpose(w, (2, 0, 1)).reshape(16, self.NB * 64)
            self.srcw[c] = np.tile(w, (8, 1))

    def key(self):
        return (
            self.N, self.F, self.H, self.C,
            tuple(self.chunks),
        )


def build_program(plan: Plan, k_iters: int = 1):
    """Build the single SPMD Bass program shared by all 8 cores."""
    F, H, C = plan.F, plan.H, plan.C
    NPC, NPAD = plan.NPC, plan.NPAD
    assert F == P and H == 2 * P

    nc = bacc.Bacc("TRN2", target_bir_lowering=False, debug=False, num_devices=NC)

    xc = nc.dram_tensor("xc", [NPC, F], FP32, kind="ExternalInput")
    W1 = nc.dram_tensor("W1", [F, H], FP32, kind="ExternalInput")
    W2 = nc.dram_tensor("W2", [H, H], FP32, kind="ExternalInput")
    Wd = nc.dram_tensor("Wd", [H, C], FP32, kind="ExternalInput")
    b1b = nc.dram_tensor("b1b", [P, H], FP32, kind="ExternalInput")
    b2b = nc.dram_tensor("b2b", [P, H], FP32, kind="ExternalInput")
    bdb = nc.dram_tensor("bdb", [P, C], FP32, kind="ExternalInput")
    iota = nc.dram_tensor("iota", [P, P], BF16, kind="ExternalInput")
    lane = nc.dram_tensor("lane", [P, 1], FP32, kind="ExternalInput")
    iotaf = nc.dram_tensor("iotaf", [P, P], FP32, kind="ExternalInput")
    dstv = nc.dram_tensor("dstv", [P, plan.NCH], FP32, kind="ExternalInput")
    srcw = nc.dram_tensor("srcw", [P, plan.NB * 64], I16, kind="ExternalInput")
    outc = nc.dram_tensor("outc", [NPC, C], FP32, kind="ExternalOutput")

    with tile.TileContext(nc) as tc:
        with (
            tc.tile_pool(name="res", bufs=1) as res,
            tc.tile_pool(name="dram", bufs=1, space="DRAM") as dram,
        ):
            nc.gpsimd.load_library(mlp)

            r = {}
            r["iota"] = res.tile([P, P], BF16, name="iota_sb")
            nc.sync.dma_start(r["iota"][:], iota[:])
            r["dstv"] = res.tile([P, plan.NCH], FP32, name="dstv_sb")
            nc.sync.dma_start(r["dstv"][:], dstv[:])
            r["srcw"] = res.tile([P, plan.NB * 64], I16, name="srcw_sb")
            nc.sync.dma_start(r["srcw"][:], srcw[:])
            r["W1"] = res.tile([P, H], FP32, name="W1_sb")
            nc.sync.dma_start(r["W1"][:], W1[:])
            r["W2a"] = res.tile([P, H], FP32, name="W2a_sb")
            nc.sync.dma_start(r["W2a"][:], W2[0:P, :])
            r["W2b"] = res.tile([P, H], FP32, name="W2b_sb")
            nc.sync.dma_start(r["W2b"][:], W2[P : 2 * P, :])
            r["Wda"] = res.tile([P, C], FP32, name="Wda_sb")
            nc.sync.dma_start(r["Wda"][:], Wd[0:P, :])
            r["Wdb"] = res.tile([P, C], FP32, name="Wdb_sb")
            nc.sync.dma_start(r["Wdb"][:], Wd[P : 2 * P, :])
            r["b1"] = res.tile([P, H], FP32, name="b1_sb")
            nc.sync.dma_start(r["b1"][:], b1b[:])
            r["b2"] = res.tile([P, H], FP32, name="b2_sb")
            nc.sync.dma_start(r["b2"][:], b2b[:])
            r["bd"] = res.tile([P, C], FP32, name="bd_sb")
            nc.sync.dma_start(r["bd"][:], bdb[:])
            r["ones"] = res.tile([P, 1], BF16, name="ones_sb")
            nc.vector.memset(r["ones"][:], 1.0)
            # fp32 identity for PE transpose: ident[p, j] = (j == p)
            lane_sb = res.tile([P, 1], FP32, name="lane_sb")
            nc.sync.dma_start(lane_sb[:], lane[:])
            iotaf_sb = res.tile([P, P], FP32, name="iotaf_sb")
            nc.sync.dma_start(iotaf_sb[:], iotaf[:])
            r["ident"] = res.tile([P, P], FP32, name="ident_sb")
            nc.vector.tensor_scalar(
                out=r["ident"][:], in0=iotaf_sb[:], scalar1=lane_sb[:],
                scalar2=None, op0=mybir.AluOpType.is_equal,
            )
            r["invs"] = res.tile([P, plan.NT], FP32, name="invs_sb")

            r["xhl_c"] = dram.tile([NPC, 2 * F], BF16, name="xhl_c")
            r["h1hl_c"] = dram.tile([NPC, 2 * H], BF16, name="h1hl_c")

            for _it in range(k_iters):
                r["xhl_full"] = dram.tile(
                    [NPAD, 2 * F], BF16, addr_space="Shared",
                    name=f"xhl_full_{_it}", tag=f"xhl_full_{_it}",
                )
                r["h1hl_full"] = dram.tile(
                    [NPAD, 2 * H], BF16, addr_space="Shared",
                    name=f"h1hl_full_{_it}", tag=f"h1hl_full_{_it}",
                )
                _emit_iter(nc, tc, plan, xc, outc, r)
    nc.compile()
    return nc


def _onehot(nc, pool, r, ci):
    oh = pool.tile([P, P], BF16, name="oh")
    nc.vector.tensor_scalar(
        out=oh[:],
        in0=r["iota"][:],
        scalar1=r["dstv"][:, ci : ci + 1],
        scalar2=None,
        op0=mybir.AluOpType.is_equal,
    )
    return oh


def _emit_iter(nc, tc, plan, xc, outc, r):
    F, H = plan.F, plan.H
    NT = plan.NT
    RG = [list(range(NC))]

    # ---------------- Phase 0: degree -> invs ----------------
    with (
        tc.tile_pool(name="p0", bufs=4) as p0,
        tc.tile_pool(name="p0ps", bufs=2, space="PSUM") as p0ps,
    ):
        deg_sb = p0.tile([P, NT], FP32, name="deg", bufs=1)
        nc.vector.memset(deg_sb[:], 0.0)
        for t in range(NT):
            cis = plan.tile_chunks[t]
            if not cis:
                continue
            psd = p0ps.tile([P, 1], FP32, name="psd")
            for k, ci in enumerate(cis):
                oh = _onehot(nc, p0, r, ci)
                nc.tensor.matmul(
                    out=psd[:], lhsT=oh[:], rhs=r["ones"][:],
                    start=(k == 0), stop=(k == len(cis) - 1),
                )
            nc.vector.tensor_copy(deg_sb[:, t : t + 1], psd[:])
        degc = p0.tile([P, NT], FP32, name="degc", bufs=1)
        nc.vector.tensor_scalar_max(degc[:], deg_sb[:], 1.0)
        recip = p0.tile([P, NT], FP32, name="recip", bufs=1)
        nc.vector.reciprocal(recip[:], degc[:])
        nc.scalar.sqrt(r["invs"][:], recip[:])

    # ---------------- Phase 1: x' = x*invs, hi/lo split, AllGather ----------
    with tc.tile_pool(name="p1", bufs=4) as p1:
        for t in range(NT):
            xt = p1.tile([P, F], FP32, name="xt")
            nc.sync.dma_start(xt[:], xc[t * P : (t + 1) * P, :])
            xs = p1.tile([P, F], FP32, name="xs")
            nc.vector.tensor_scalar_mul(xs[:], xt[:], r["invs"][:, t : t + 1])
            hl = p1.tile([P, 2 * F], BF16, name="xhlt")
            nc.scalar.copy(hl[:, 0:F], xs[:])
            nc.vector.tensor_sub(hl[:, F : 2 * F], xs[:], hl[:, 0:F])
            nc.sync.dma_start(r["xhl_c"][t * P : (t + 1) * P, :], hl[:])
        nc.gpsimd.collective_compute(
            "AllGather", mybir.AluOpType.bypass, replica_groups=RG,
            ins=[r["xhl_c"][:].opt()], outs=[r["xhl_full"][:].opt()],
        )

    # ---------------- Phase 2: layer 1 ----------------
    def dense1(pool, psum, psum_tr, aggT_list, t):
        ph = psum.tile([P, H], FP32, name="ph1")
        nc.tensor.matmul(
            out=ph[:], lhsT=aggT_list[0][:], rhs=r["W1"][:],
            start=True, stop=True,
        )
        z = pool.tile([P, H], FP32, name="z1")
        nc.vector.tensor_add(z[:], ph[:], r["b1"][:])
        h1p = pool.tile([P, H], FP32, name="h1p")
        nc.scalar.activation(
            h1p[:], z[:], mybir.ActivationFunctionType.Relu,
            scale=r["invs"][:, t : t + 1],
        )
        hl = pool.tile([P, 2 * H], BF16, name="h1hlt")
        nc.scalar.copy(hl[:, 0:H], h1p[:])
        nc.vector.tensor_sub(hl[:, H : 2 * H], h1p[:], hl[:, 0:H])
        nc.sync.dma_start(r["h1hl_c"][t * P : (t + 1) * P, :], hl[:])

    _graph_layer(nc, tc, plan, r, table=r["xhl_full"], feat=F,
                 dense=dense1, phase="L1")

    with tc.tile_pool(name="agp", bufs=1):
        nc.gpsimd.collective_compute(
            "AllGather", mybir.AluOpType.bypass, replica_groups=RG,
            ins=[r["h1hl_c"][:].opt()], outs=[r["h1hl_full"][:].opt()],
        )

    # ---------------- Phase 3: layer 2 + head + softmax ----------------
    def dense2(pool, psum, psum_tr, aggT_list, t):
        C = plan.C
        ph = psum.tile([P, H], FP32, name="ph2", tag="ph")
        nc.tensor.matmul(out=ph[:], lhsT=aggT_list[0][:], rhs=r["W2a"][:],
                         start=True, stop=False)
        nc.tensor.matmul(out=ph[:], lhsT=aggT_list[1][:], rhs=r["W2b"][:],
                         start=False, stop=True)
        z = pool.tile([P, H], FP32, name="z2")
        nc.vector.tensor_add(z[:], ph[:], r["b2"][:])
        h2 = pool.tile([P, H], FP32, name="h2")
        nc.scalar.activation(h2[:], z[:], mybir.ActivationFunctionType.Relu)
        h2T = []
        for hb in range(2):
            ptr = psum_tr.tile([P, P], FP32, name=f"ptrh{hb}", tag="ptr")
            nc.tensor.transpose(
                out=ptr[:], in_=h2[:, hb * P : (hb + 1) * P],
                identity=r["ident"][:],
            )
            hT = pool.tile([P, P], FP32, name=f"h2T{hb}")
            nc.vector.tensor_copy(hT[:], ptr[:])
            h2T.append(hT)
        po = psum.tile([P, C], FP32, name="po", tag="ph")
        nc.tensor.matmul(out=po[:], lhsT=h2T[0][:], rhs=r["Wda"][:],
                         start=True, stop=False)
        nc.tensor.matmul(out=po[:], lhsT=h2T[1][:], rhs=r["Wdb"][:],
                         start=False, stop=True)
        zl = pool.tile([P, C], FP32, name="zl")
        nc.vector.tensor_add(zl[:], po[:], r["bd"][:])
        m = pool.tile([P, 1], FP32, name="m")
        nc.vector.reduce_max(m[:], zl[:], axis=mybir.AxisListType.X)
        negm = pool.tile([P, 1], FP32, name="negm")
        nc.vector.tensor_scalar_mul(negm[:], m[:], -1.0)
        ex = pool.tile([P, C], FP32, name="ex")
        ssum = pool.tile([P, 1], FP32, name="ssum")
        nc.scalar.activation(
            ex[:], zl[:], mybir.ActivationFunctionType.Exp,
            bias=negm[:], accum_out=ssum[:],
        )
        rs = pool.tile([P, 1], FP32, name="rs")
        nc.vector.reciprocal(rs[:], ssum[:])
        o = pool.tile([P, C], FP32, name="o")
        nc.vector.tensor_scalar_mul(o[:], ex[:], rs[:])
        nc.sync.dma_start(outc[t * P : (t + 1) * P, :], o[:])

    _graph_layer(nc, tc, plan, r, table=r["h1hl_full"], feat=H,
                 dense=dense2, phase="L2")


def _graph_layer(nc, tc, plan, r, *, table, feat, dense, phase):
    """Gather + one-hot scatter-matmul + per-tile dense epilogue."""
    HALFR = plan.HALF
    row = 2 * feat
    half_view = [table[0:HALFR, :], table[HALFR : 2 * HALFR, :]]
    with (
        tc.tile_pool(name=f"{phase}g", bufs=6) as gp,
        tc.tile_pool(name=f"{phase}w", bufs=4) as wp,
        tc.tile_pool(name=f"{phase}ps", bufs=2, space="PSUM") as psp,
        tc.tile_pool(name=f"{phase}tr", bufs=2, space="PSUM") as trp,
        tc.tile_pool(name=f"{phase}dn", bufs=2, space="PSUM") as dnp,
    ):
        gathered = {}
        for t in range(plan.NT):
            cis = plan.tile_chunks[t]
            aggn = wp.tile([P, feat], FP32, name="aggn")
            if cis:
                ps = psp.tile([P, feat], FP32, name="psagg")
                for k, ci in enumerate(cis):
                    (tt, h, b, pos) = plan.chunks[ci]
                    if b not in gathered:
                        g = gp.tile([P, BATCH, row], BF16, name="gt")
                        nc.gpsimd.dma_gather(
                            g[:], half_view[h],
                            r["srcw"][:, b * 64 : (b + 1) * 64],
                            BATCH * P, BATCH * P, row,
                        )
                        gathered[b] = g
                    g = gathered[b]
                    oh = _onehot(nc, wp, r, ci)
                    nc.tensor.matmul(
                        out=ps[:], lhsT=oh[:], rhs=g[:, pos, 0:feat],
                        start=(k == 0), stop=False,
                    )
                    nc.tensor.matmul(
                        out=ps[:], lhsT=oh[:], rhs=g[:, pos, feat : 2 * feat],
                        start=False, stop=(k == len(cis) - 1),
                    )
                nc.vector.tensor_scalar_mul(
                    aggn[:], ps[:], r["invs"][:, t : t + 1]
                )
            else:
                nc.vector.memset(aggn[:], 0.0)
            aggT_list = []
            for hb in range(feat // P):
                ptr = trp.tile([P, P], FP32, name="ptr")
                nc.tensor.transpose(
                    out=ptr[:], in_=aggn[:, hb * P : (hb + 1) * P],
                    identity=r["ident"][:],
                )
                aggT = wp.tile([P, P], FP32, name=f"aggT{hb}")
                nc.vector.tensor_copy(aggT[:], ptr[:])
                aggT_list.append(aggT)
            dense(wp, dnp, trp, aggT_list, t)


# ---------------------------------------------------------------------------
_CACHE = {}
K_ITERS = 1


def _get_program(plan, k_iters):
    key = plan.key() + (k_iters,)
    if key not in _CACHE:
        _CACHE[key] = build_program(plan, k_iters)
    return _CACHE[key]


def make_in_maps(plan, x, W1, b1, W2, b2, Wd, bd):
    N, F = x.shape
    H = W1.shape[1]
    C = Wd.shape[1]
    iota_b = np.broadcast_to(
        np.arange(P, dtype=np.float32), (P, P)
    ).astype(ml_dtypes.bfloat16)
    iota_f = np.broadcast_to(np.arange(P, dtype=np.float32), (P, P))
    lane_f = np.arange(P, dtype=np.float32).reshape(P, 1)
    in_maps = []
    for c in range(NC):
        lo = c * plan.NPC
        hi = min(lo + plan.NPC, N)
        xcc = np.zeros((plan.NPC, F), dtype=np.float32)
        xcc[: hi - lo] = x[lo:hi]
        in_maps.append(
            {
                "xc": xcc,
                "W1": W1,
                "W2": W2,
                "Wd": Wd,
                "b1b": np.broadcast_to(b1, (P, H)).copy(),
                "b2b": np.broadcast_to(b2, (P, H)).copy(),
                "bdb": np.broadcast_to(bd, (P, C)).copy(),
                "iota": np.ascontiguousarray(iota_b),
                "iotaf": np.ascontiguousarray(iota_f),
                "lane": lane_f,
                "dstv": np.ascontiguousarray(plan.dstv[c]),
                "srcw": np.ascontiguousarray(plan.srcw[c]),
            }
        )
    return in_maps


def kernel(x, edge_index, W1, b1, W2, b2, Wd, bd):
    x = np.asarray(x, dtype=np.float32)
    edge_index = np.asarray(edge_index, dtype=np.int32)
    W1 = np.asarray(W1, dtype=np.float32)
    b1 = np.asarray(b1, dtype=np.float32)
    W2 = np.asarray(W2, dtype=np.float32)
    b2 = np.asarray(b2, dtype=np.float32)
    Wd = np.asarray(Wd, dtype=np.float32)
    bd = np.asarray(bd, dtype=np.float32)

    N, F = x.shape
    H = W1.shape[1]
    C = Wd.shape[1]
    plan = Plan(N, F, H, C, edge_index)
    nc = _get_program(plan, K_ITERS)
    in_maps = make_in_maps(plan, x, W1, b1, W2, b2, Wd, bd)

    res = run_bass_kernel_spmd(nc, in_maps, core_ids=list(range(NC)))
    out = np.empty((N, C), dtype=np.float32)
    for c in range(NC):
        lo = c * plan.NPC
        hi = min(lo + plan.NPC, N)
        out[lo:hi] = res.results[c]["outc"][: hi - lo]
    return out
